# revision 17
# baseline (speedup 1.0000x reference)
"""CircleLoss on 8 Trainium2 NeuronCores.

Math (reference):
    f = l2_normalize(features)              # (4096, 512)
    sim = f @ f.T                           # (4096, 4096), sim in [-1, 1]
    pos_term = -relu(1 + M - sim) * sim * G # M=0.25, G=256
    neg_term =  relu(sim + M) * sim * G
    loss = softplus(lse(pos_term | same-label) + lse(neg_term | diff-label))

Key identities used on device (exact, since sim <= 1 so relu(1.25-sim) is
always active):
    pos_term = 256*s^2 - 320*s           = 256*(s - 0.625)^2 - 100
    neg_term = 256*relu(s+0.25)*s        = 256*(u - 0.125)^2 - 4,  u = relu(s+0.25)

Sharding: core c owns rows [c*512, (c+1)*512) of sim. Each core receives the
full (rotated) normalized feature matrix transposed [512, 4096]; its local
block is always columns [0, 512) of the rotated matrix, so the kernel is pure
SPMD with static offsets. Per row the kernel emits (rowmax, sum_exp) for the
pos and neg streams; the host does the exact logsumexp combine (the "tiny
all-reduce") and the final softplus.

Masking: mask = (label_i == label_j) in {0,1}. posq = sqp + 6*mask,
negq = sqn - 6*mask with term = 256*q - const, so masked-out entries sit
~1536 below the valid range and vanish in exp(256*(q - rowmax)).
"""

import numpy as np
from contextlib import ExitStack

N = 4096
D = 512
NCORES = 8
ROWS_PER_CORE = N // NCORES          # 512
RT = ROWS_PER_CORE // 128            # 4 row-tiles per core
NC_CHUNK = 512                       # free-dim chunk (1 PSUM bank)
NCHUNK = N // NC_CHUNK               # 8 chunks
KT = D // 128                        # 4 k-tiles
BIGQ = 6.0                           # mask offset in q-units (256*6 = 1536)

_CACHE = {}

# Set by test.py to request a profiled run; kernel() stores the spmd result
# object here so the harness can read exec_time_ns / trace paths.
TRACE = False
LAST_RESULT = None


def _build_nc():
    import concourse.bass as bass
    import concourse.bacc as bacc
    import concourse.tile as tile
    from concourse import mybir

    f32 = mybir.dt.float32
    AF = mybir.ActivationFunctionType
    ALU = mybir.AluOpType
    AX = mybir.AxisListType

    # Bacc (not plain Bass): its finalize() runs move_matmul_waits_to_ldweights
    # + generate_event_semaphores, required on TRN2 (1 sync wait per inst).
    f32r = mybir.dt.float32r
    nc = bacc.Bacc(None)
    # ft declared float32r end-to-end (bit-identical to f32 in memory) so the
    # fp32r matmuls pass BIR verification; PE runs them at full (1 cyc/row) rate
    ft_h = nc.dram_tensor("ft", [D, N], f32, kind="ExternalInput")
    lab_h = nc.dram_tensor("lab", [N], f32, kind="ExternalInput")
    stats_h = nc.dram_tensor("stats", [128, 4 * RT], f32, kind="ExternalOutput")

    ft_v = ft_h[:].rearrange("(kt p) n -> kt p n", p=128)   # [KT, 128, N]

    with tile.TileContext(nc) as tc, ExitStack() as ctx:
        persist = ctx.enter_context(tc.tile_pool(name="persist", bufs=1))
        rowt = ctx.enter_context(tc.tile_pool(name="rowt", bufs=2))
        maskp = ctx.enter_context(tc.tile_pool(name="maskp", bufs=1))
        negtp = ctx.enter_context(tc.tile_pool(name="negtp", bufs=3))
        sm = ctx.enter_context(tc.tile_pool(name="sm", bufs=4))
        ps = ctx.enter_context(tc.tile_pool(name="ps", bufs=4, space="PSUM"))

        # --- load the full transposed normalized features (8 MB) ---
        ft_t = []
        for k in range(KT):
            t = persist.tile([128, N], f32, tag=f"ft{k}")
            nc.sync.dma_start(out=t[:], in_=ft_v[k])
            ft_t.append(t)

        # --- labels: [1, N] row, then broadcast to all 128 partitions ---
        lab_row = maskp.tile([1, N], f32, tag="mask")
        nc.sync.dma_start(out=lab_row[:], in_=lab_h[:].rearrange("(o n) -> o n", o=1))
        ones_t = persist.tile([1, 128], f32, tag="ones")
        nc.vector.memset(ones_t[:], 1.0)
        lab_bcast = persist.tile([128, N], f32, tag="labbc")
        for c in range(NCHUNK):
            pt = ps.tile([128, NC_CHUNK], f32, tag="ps")
            sl = slice(c * NC_CHUNK, (c + 1) * NC_CHUNK)
            nc.tensor.matmul(pt[:], ones_t[:], lab_row[:, sl], start=True, stop=True)
            nc.scalar.copy(lab_bcast[:, sl], pt[:])

        # --- per-row-tile local labels [128, 1] ---
        lab_loc = []
        for t in range(RT):
            lt = sm.tile([128, 1], f32, tag=f"labloc{t}")
            nc.sync.dma_start(
                out=lt[:],
                in_=lab_h[:][t * 128:(t + 1) * 128].rearrange("(p o) -> p o", o=1),
            )
            lab_loc.append(lt)

        stats_t = persist.tile([128, 4 * RT], f32, tag="stats")

        # constant per-partition bias tiles for activation ops
        def const_col(val, tag):
            t = sm.tile([128, 1], f32, tag=tag)
            nc.vector.memset(t[:], val)
            return t

        b_sqp = const_col(-0.625, "b_sqp")
        b_u = const_col(0.25, "b_u")
        b_sqn = const_col(-0.125, "b_sqn")

        for t in range(RT):
            mask = maskp.tile([128, N], f32, tag="mask")
            nc.vector.tensor_scalar(mask[:], lab_bcast[:], lab_loc[t][:], None,
                                    op0=ALU.is_equal)
            posq = rowt.tile([128, N], f32, tag="sqp_rt")
            negq = rowt.tile([128, N], f32, tag="u_rt")

            for c in range(NCHUNK):
                sl = slice(c * NC_CHUNK, (c + 1) * NC_CHUNK)
                pt = ps.tile([128, NC_CHUNK], f32, tag="ps")
                for k in range(KT):
                    nc.tensor.matmul(
                        pt[:],
                        ft_t[k][:, t * 128:(t + 1) * 128],
                        ft_t[k][:, sl],
                        start=(k == 0),
                        stop=(k == KT - 1),
                    )
                sqp = negtp.tile([128, NC_CHUNK], f32, tag="sqp")
                nc.scalar.activation(sqp[:], pt[:], AF.Square, bias=b_sqp[:])
                u = negtp.tile([128, NC_CHUNK], f32, tag="u")
                nc.scalar.activation(u[:], pt[:], AF.Relu, bias=b_u[:])
                sqn = negtp.tile([128, NC_CHUNK], f32, tag="sqn")
                nc.scalar.activation(sqn[:], u[:], AF.Square, bias=b_sqn[:])
                nc.vector.scalar_tensor_tensor(
                    posq[:, sl], mask[:, sl], BIGQ, sqp[:],
                    op0=ALU.mult, op1=ALU.add,
                )
                nc.vector.scalar_tensor_tensor(
                    negq[:, sl], mask[:, sl], -BIGQ, sqn[:],
                    op0=ALU.mult, op1=ALU.add,
                )

            mp = sm.tile([128, 1], f32, tag="mp")
            mn = sm.tile([128, 1], f32, tag="mn")
            nc.vector.reduce_max(mp[:], posq[:], axis=AX.X)
            nc.vector.reduce_max(mn[:], negq[:], axis=AX.X)
            biasp = sm.tile([128, 1], f32, tag="biasp")
            biasn = sm.tile([128, 1], f32, tag="biasn")
            nc.vector.tensor_scalar(biasp[:], mp[:], -256.0, None, op0=ALU.mult)
            nc.vector.tensor_scalar(biasn[:], mn[:], -256.0, None, op0=ALU.mult)
            sp = sm.tile([128, 1], f32, tag="sp")
            sn = sm.tile([128, 1], f32, tag="sn")
            nc.scalar.activation(posq[:], posq[:], AF.Exp, bias=biasp[:],
                                 scale=256.0, accum_out=sp[:])
            nc.scalar.activation(negq[:], negq[:], AF.Exp, bias=biasn[:],
                                 scale=256.0, accum_out=sn[:])
            nc.vector.tensor_copy(stats_t[:, t:t + 1], mp[:])
            nc.vector.tensor_copy(stats_t[:, RT + t:RT + t + 1], mn[:])
            nc.vector.tensor_copy(stats_t[:, 2 * RT + t:2 * RT + t + 1], sp[:])
            nc.vector.tensor_copy(stats_t[:, 3 * RT + t:3 * RT + t + 1], sn[:])

        nc.sync.dma_start(out=stats_h[:], in_=stats_t[:])

    nc.finalize()
    return nc


def _get_nc():
    if "nc" not in _CACHE:
        _CACHE["nc"] = _build_nc()
    return _CACHE["nc"]


def _prep_inputs(features, labels):
    feats = np.asarray(features, dtype=np.float32)
    lab = np.asarray(labels).astype(np.float32)
    nrm = np.sqrt((feats.astype(np.float64) ** 2).sum(axis=1))
    nrm = np.maximum(nrm, 1e-12)
    f = (feats / nrm[:, None].astype(np.float32)).astype(np.float32)
    fT = np.ascontiguousarray(f.T)  # [D, N]
    in_maps = []
    for c in range(NCORES):
        sh = c * ROWS_PER_CORE
        in_maps.append({
            "ft": np.ascontiguousarray(np.roll(fT, -sh, axis=1)),
            "lab": np.ascontiguousarray(np.roll(lab, -sh)),
        })
    return in_maps


def _combine(stats_list):
    """Exact logsumexp combine from per-row (max, sumexp) stats."""
    mp, mn, sp, sn = [], [], [], []
    for st in stats_list:  # st: [128, 16]
        mp.append(st[:, 0:RT].T.reshape(-1))
        mn.append(st[:, RT:2 * RT].T.reshape(-1))
        sp.append(st[:, 2 * RT:3 * RT].T.reshape(-1))
        sn.append(st[:, 3 * RT:4 * RT].T.reshape(-1))
    mp = np.concatenate(mp).astype(np.float64)
    mn = np.concatenate(mn).astype(np.float64)
    sp = np.concatenate(sp).astype(np.float64)
    sn = np.concatenate(sn).astype(np.float64)

    # true row maxes: pos includes the +1536 mask offset
    Mp = 256.0 * mp - 100.0
    Mn = 256.0 * mn - 4.0

    def lse(M, S):
        g = M.max()
        return g + np.log((S * np.exp(M - g)).sum())

    lse_pos = lse(Mp, sp) - 256.0 * BIGQ
    lse_neg = lse(Mn, sn)
    loss = np.logaddexp(0.0, lse_pos + lse_neg)
    return np.asarray(loss, dtype=np.float32)


def kernel(features, labels):
    global LAST_RESULT
    from concourse.bass_utils import run_bass_kernel_spmd

    nc = _get_nc()
    in_maps = _prep_inputs(features, labels)
    res = run_bass_kernel_spmd(
        nc, in_maps, core_ids=list(range(NCORES)), trace=TRACE,
    )
    LAST_RESULT = res
    stats_list = [res.results[c]["stats"] for c in range(NCORES)]
    return _combine(stats_list)
